# revision 1
# baseline (speedup 1.0000x reference)
"""Batched MHA (paged decode + packed varlen prefill) on 8 Trainium2 cores.

Sharding: tensor-parallel over heads (16 heads -> 2 per core).
  - w_q/w_k/w_v column-sharded (each core computes Q/K/V for its 2 heads,
    for all tokens), w_o row-sharded (each core emits a full-shape partial
    output; host sums the 8 partials).
  - k/v cache: each core gets the 2-head slice of the decode slots, host
    pre-transposed (K) / pre-tiled (V) and cast to bf16 so every device DMA
    is large and per-partition contiguous.

All matmuls run on the PE in bf16 (fp32 accumulate in PSUM); softmax runs
without max-subtraction (scores are O(1) by construction), exp on ACT in
fp32 with the 1/sqrt(dh) scale folded in.
"""

import math
from functools import lru_cache

import ml_dtypes
import numpy as np

BF16 = ml_dtypes.bfloat16

H = 16          # total heads
DH = 128        # head dim
NCORES = 8
HPC = H // NCORES  # heads per core = 2
SCALE = 1.0 / math.sqrt(DH)
_ABLATE = frozenset()   # dev-only: {'decode','prefill','qkv','oproj'} to skip phases


def _ceil_div(a, b):
    return (a + b - 1) // b


@lru_cache(maxsize=4)
def _build_program(nt, hid, L, nd, dec_lens, pre_ranges):
    """Build + compile the SPMD Bass program (identical on all cores).

    dec_lens: tuple of nd ints (cache write position / #old positions per seq)
    pre_ranges: tuple of (tok0, tok1) global token ranges, one per prefill seq
    """
    import concourse.bacc as bacc
    import concourse.mybir as mybir
    import concourse.tile as tile

    fp32 = mybir.dt.float32
    bf16 = mybir.dt.bfloat16
    Exp = mybir.ActivationFunctionType.Exp
    X = mybir.AxisListType.X
    mult = mybir.AluOpType.mult
    add = mybir.AluOpType.add

    KHID = hid // 128          # 16 k-tiles
    HD = HPC * DH              # 256 head dims per core
    LT = L // 128              # 32 cache tiles max

    nc = bacc.Bacc("TRN2", target_bir_lowering=False, debug=False,
                   num_devices=NCORES)

    xT = nc.dram_tensor("xT", [hid, nt], bf16, kind="ExternalInput")
    # w*_t[p, k*HD + m] = W[c*HD + m, k*128 + p]   (per-core, host-tiled)
    wq_t = nc.dram_tensor("wq_t", [128, KHID * HD], bf16, kind="ExternalInput")
    wk_t = nc.dram_tensor("wk_t", [128, KHID * HD], bf16, kind="ExternalInput")
    wv_t = nc.dram_tensor("wv_t", [128, KHID * HD], bf16, kind="ExternalInput")
    # woT[m, :] = w_o[:, c*HD + m]
    woT = nc.dram_tensor("woT", [HD, hid], bf16, kind="ExternalInput")
    tri = nc.dram_tensor("tri", [128, 128], bf16, kind="ExternalInput")
    out_p = nc.dram_tensor("out_partial", [nt, hid], bf16, kind="ExternalOutput")
    if nd > 0:
        # ktc[n, j] = k_cache[idx_n, 2c+j].T        [128(dh), L]
        ktc = nc.dram_tensor("ktc", [nd, HPC, DH, L], bf16, kind="ExternalInput")
        # vtc[n, j, p, t, d] = v_cache[idx_n, 2c+j, t*128+p, d]
        vtc = nc.dram_tensor("vtc", [nd, HPC, 128, LT, DH], bf16,
                             kind="ExternalInput")

    ntt = _ceil_div(nt, 128)   # token tiles (0-aligned) for O-proj

    with tile.TileContext(nc) as tc:
        from contextlib import ExitStack
        with ExitStack() as ctx:
            const_pool = ctx.enter_context(tc.tile_pool(name="const", bufs=1))
            xw_pool = ctx.enter_context(tc.tile_pool(name="xw", bufs=1))
            proj_pool = ctx.enter_context(tc.tile_pool(name="proj", bufs=1))
            ps_pool = ctx.enter_context(
                tc.tile_pool(name="ps_pool", bufs=1, space="PSUM"))
            cache_pool = ctx.enter_context(tc.tile_pool(name="cache", bufs=3))
            dec_sb = ctx.enter_context(tc.tile_pool(name="dec_sb", bufs=4))
            est_pool = ctx.enter_context(tc.tile_pool(name="est", bufs=8))
            nrm_pool = ctx.enter_context(tc.tile_pool(name="nrm", bufs=4))
            o_sb = ctx.enter_context(tc.tile_pool(name="o_sb", bufs=4))

            # ---- constants ----
            tri_sb = const_pool.tile([128, 128], bf16)
            nc.gpsimd.dma_start(out=tri_sb[:], in_=tri[:])
            ones_b = const_pool.tile([128, 1], bf16)   # bf16 ones column
            nc.gpsimd.memset(ones_b[:], 1.0)
            ones_rf = const_pool.tile([1, 128], fp32)  # f32 ones row
            nc.gpsimd.memset(ones_rf[:], 1.0)
            ones_rb = const_pool.tile([1, 128], bf16)  # bf16 ones row
            nc.gpsimd.memset(ones_rb[:], 1.0)

            # ---- load weights first, then xT tiles on two HWDGE rings ----
            w_sb = {}
            for name, dram in (("q", wq_t), ("k", wk_t), ("v", wv_t)):
                t = xw_pool.tile([128, KHID * HD], bf16, tag=f"w{name}")
                nc.gpsimd.dma_start(out=t[:], in_=dram[:])
                w_sb[name] = t
            xt_sb = []
            xh = min(1024, nt)
            for k in range(KHID):
                t = xw_pool.tile([128, nt], bf16, tag=f"xt{k}")
                eng = nc.sync if k % 2 == 0 else nc.scalar
                eng.dma_start(out=t[:, 0:xh],
                              in_=xT[k * 128:(k + 1) * 128, 0:xh])
                xt_sb.append(t)
            for k in range(KHID):
                if xh < nt:
                    eng = nc.sync if k % 2 == 0 else nc.scalar
                    eng.dma_start(out=xt_sb[k][:, xh:nt],
                                  in_=xT[k * 128:(k + 1) * 128, xh:nt])
            woT_sb = []
            for j in range(HPC):
                t = xw_pool.tile([128, hid], bf16, tag=f"wo{j}")
                nc.gpsimd.dma_start(out=t[:], in_=woT[j * 128:(j + 1) * 128, :])
                woT_sb.append(t)

            # ---- QKV projections ----
            # Q.T / K.T : [128, nt] per head-half, from lhsT=w, rhs=xT
            QT_sb = [proj_pool.tile([128, nt], bf16, tag=f"qT{j}", name=f"qT{j}")
                     for j in range(HPC)]
            KT_sb = [proj_pool.tile([128, nt], bf16, tag=f"kT{j}", name=f"kT{j}")
                     for j in range(HPC)]
            # blocks: [0, nd) decode tokens first (tiny -> decode units can
            # start early), then 512-wide prefill-aligned blocks
            qk_blocks = ([(0, nd)] if nd > 0 else [])
            qk_blocks += [(b0, min(b0 + 512, nt))
                          for b0 in range(nd, nt, 512)]

            def _emit_qk_block(b):
                b0, b1 = qk_blocks[b]
                for name, dest in (("q", QT_sb), ("k", KT_sb)):
                    for j in range(HPC):
                        ps = ps_pool.tile([128, 512], fp32, tag="ps_qk",
                                          bufs=2, name="ps_qk")
                        for k in range(KHID):
                            nc.tensor.matmul(
                                ps[:, 0:b1 - b0],
                                w_sb[name][:, k * HD + j * 128:
                                           k * HD + (j + 1) * 128],
                                xt_sb[k][:, b0:b1],
                                start=(k == 0), stop=(k == KHID - 1))
                        nc.vector.tensor_copy(dest[j][:, b0:b1],
                                              ps[:, 0:b1 - b0])

            _emit_qk_block(0)

            # V natural, tiled per prefill seq (seq-local 128 grids) + decode
            def v_block(tok0, tok1, tag):
                """compute V[tok0:tok1, :] into a [128, nkt*HD] bf16 tile"""
                lsz = tok1 - tok0
                nkt = _ceil_div(lsz, 128)
                vt = proj_pool.tile([128, nkt * HD], bf16, tag=tag, name=tag)
                for t in range(nkt):
                    t0 = tok0 + t * 128
                    tw = min(128, tok1 - t0)
                    ps = ps_pool.tile([128, HD], fp32, tag="ps_v", bufs=1, name="ps_v")
                    for k in range(KHID):
                        nc.tensor.matmul(
                            ps[0:tw, :],
                            xt_sb[k][:, t0:t0 + tw],
                            w_sb["v"][:, k * HD:(k + 1) * HD],
                            start=(k == 0), stop=(k == KHID - 1))
                    if tw < 128:
                        nc.vector.memset(vt[:, t * HD:(t + 1) * HD], 0.0)
                    nc.scalar.copy(vt[0:tw, t * HD:(t + 1) * HD], ps[0:tw, :])
                return vt

            V_dec = v_block(0, nd, "v_dec") if nd > 0 else None
            # decode V rows re-staged at partition 0 (matmul lhsT needs base 0)
            vnew_sb = None
            if nd > 0:
                vnew_sb = proj_pool.tile([1, nd * HPC * DH], bf16,
                                         name="vnew_sb")
                for n in range(nd):
                    for j in range(HPC):
                        nc.sync.dma_start(
                            out=vnew_sb[0:1, (n * HPC + j) * DH:
                                        (n * HPC + j + 1) * DH],
                            in_=V_dec[n:n + 1, j * DH:(j + 1) * DH])
            V_pre = {}

            # attention output (transposed) per head-half
            attnT = [[proj_pool.tile([128, 128], bf16, tag=f"aT{j}_{tt}",
                                     name=f"aT{j}_{tt}")
                      for tt in range(ntt)] for j in range(HPC)]

            _oproj_pending = set(range(ntt))

            def _emit_oproj(tt, late=False):
                t0 = tt * 128
                tw = min(128, nt - t0)
                for nb in range(hid // 512):
                    if late and nb % 2 == 0:
                        ops = ps_pool.tile([128, 512], fp32, tag="st",
                                           bufs=3, name="ops_l")
                    else:
                        ops = ps_pool.tile([128, 512], fp32, tag="ps_qk",
                                           bufs=2, name="ops")
                    for j in range(HPC):
                        nc.tensor.matmul(
                            ops[0:tw, :],
                            attnT[j][tt][:, 0:tw],
                            woT_sb[j][:, nb * 512:(nb + 1) * 512],
                            start=(j == 0), stop=(j == HPC - 1))
                    stage = o_sb.tile([128, 512], bf16, tag="stage",
                                      name="stage")
                    if late and nb % 2 == 1:
                        nc.scalar.copy(stage[0:tw, :], ops[0:tw, :])
                    else:
                        nc.vector.tensor_copy(stage[0:tw, :], ops[0:tw, :])
                    nc.sync.dma_start(
                        out=out_p[t0:t0 + tw, nb * 512:(nb + 1) * 512],
                        in_=stage[0:tw, :])

            def _flush_oproj(upto_tok, late=False):
                if 'oproj' in _ABLATE:
                    return
                for tt in sorted(_oproj_pending):
                    if (tt + 1) * 128 <= upto_tok:
                        _emit_oproj(tt, late=late)
                        _oproj_pending.discard(tt)

            # ---- decode attention ----
            def _emit_decode(n):
                ln = dec_lens[n]
                T = _ceil_div(ln, 128)
                r = ln - 128 * (T - 1) if T > 0 else 0
                for j in range(HPC):
                    dw = ps_pool.tile([128, 512], fp32, tag="dwork", bufs=1, name="dwork")
                    if T > 0:
                        kt_sb = cache_pool.tile([128, LT * 128], bf16,
                                                tag="ktc")
                        nc.gpsimd.dma_start(
                            out=kt_sb[:, 0:T * 128],
                            in_=ktc[n, j, :, 0:T * 128])
                        vt_sb = cache_pool.tile([128, LT * DH], bf16,
                                                tag="vtc")
                        nc.sync.dma_start(
                            out=vt_sb[:, 0:T * DH],
                            in_=vtc[n, j, :, 0:T, :])
                        for t in range(T):
                            nc.tensor.matmul(
                                dw[:, t:t + 1],
                                kt_sb[:, t * 128:(t + 1) * 128],
                                QT_sb[j][:, n:n + 1],
                                start=True, stop=True)
                        es = dec_sb.tile([128, LT], bf16, tag="es")
                        if r < 128:
                            nc.vector.memset(es[:, T - 1:T], 0.0)
                        if T > 1:
                            nc.scalar.activation(es[:, 0:T - 1],
                                                 dw[:, 0:T - 1],
                                                 Exp, scale=SCALE)
                        nc.scalar.activation(es[0:r, T - 1:T],
                                             dw[0:r, T - 1:T],
                                             Exp, scale=SCALE)
                    # new-token score: q . k_new
                    nc.tensor.matmul(dw[0:1, 48:49],
                                     KT_sb[j][:, n:n + 1],
                                     QT_sb[j][:, n:n + 1],
                                     start=True, stop=True)
                    esn_f = dec_sb.tile([1, 2], fp32, tag="esnf")
                    esn_b = dec_sb.tile([1, 1], bf16, tag="esnb")
                    nc.scalar.activation(esn_f[0:1, 0:1], dw[0:1, 48:49],
                                         Exp, scale=SCALE)
                    nc.scalar.activation(esn_b[0:1, 0:1], dw[0:1, 48:49],
                                         Exp, scale=SCALE)
                    # Z = sum(es) + esn
                    ztot = dec_sb.tile([1, 1], fp32, tag="ztot")
                    if T > 0:
                        nc.tensor.matmul(dw[0:1, 64:64 + T],
                                         ones_b[:], es[:, 0:T],
                                         start=True, stop=True)
                        nc.vector.reduce_sum(esn_f[0:1, 1:2],
                                             dw[0:1, 64:64 + T], axis=X)
                        nc.vector.tensor_tensor(ztot[:], esn_f[0:1, 0:1],
                                                esn_f[0:1, 1:2], op=add)
                    else:
                        nc.vector.tensor_copy(ztot[:], esn_f[0:1, 0:1])
                    rec = dec_sb.tile([1, 1], fp32, tag="rec")
                    nc.vector.reciprocal(rec[:], ztot[:])
                    nc.tensor.matmul(dw[:, 112:113], ones_rf[:],
                                     rec[:], start=True, stop=True)
                    recb = dec_sb.tile([128, 1], fp32, tag="recb")
                    nc.scalar.copy(recb[:], dw[:, 112:113])
                    # PV
                    for t in range(T):
                        nc.tensor.matmul(dw[:, 128:129],
                                         vt_sb[:, t * DH:(t + 1) * DH],
                                         es[:, t:t + 1],
                                         start=(t == 0), stop=False)
                    nc.tensor.matmul(dw[:, 128:129],
                                     vnew_sb[0:1, (n * HPC + j) * DH:
                                             (n * HPC + j + 1) * DH],
                                     esn_b[:],
                                     start=(T == 0), stop=True)
                    nc.scalar.activation(
                        attnT[j][n // 128][:, n % 128:n % 128 + 1],
                        dw[:, 128:129],
                        mybir.ActivationFunctionType.Copy,
                        scale=recb[:])

            if 'decode' not in _ABLATE:
                for n in range(nd):
                    _emit_decode(n)
            for b in range(1, len(qk_blocks)):
                _emit_qk_block(b)

            # ---- prefill attention ----
            def _emit_prefill(si, q0, q1):
                lsz = q1 - q0
                nkt = _ceil_div(lsz, 128)
                for j in range(HPC):
                    for qb in range(0, lsz, 512):
                        qw = min(512, lsz - qb)
                        nkt_b = min(nkt, _ceil_div(qb + qw, 128))
                        # Z row and PV accumulate across waves of key tiles;
                        # est tiles recycle between waves (pool has 6 slots)
                        zr = ps_pool.tile([128, 512], fp32, tag="st", bufs=3, name="zr")
                        ot = ps_pool.tile([128, 512], fp32, tag="pout", bufs=1, name="ot")
                        WAVE = 4
                        for w0 in range(0, nkt_b, WAVE):
                            wave = range(w0, min(w0 + WAVE, nkt_b))
                            ests = []
                            for kt in wave:
                                k0 = kt * 128
                                kw = min(128, lsz - k0)
                                c0 = max(0, k0 - qb)
                                stp = ps_pool.tile([128, 512], fp32, tag="st", bufs=3, name="stp")
                                nc.tensor.matmul(
                                    stp[0:kw, c0:qw],
                                    KT_sb[j][:, q0 + k0:q0 + k0 + kw],
                                    QT_sb[j][:, q0 + qb + c0:q0 + qb + qw],
                                    start=True, stop=True)
                                est = est_pool.tile([128, 512], bf16, tag="est")
                                nc.scalar.activation(est[0:kw, c0:qw],
                                                     stp[0:kw, c0:qw],
                                                     Exp, scale=SCALE)
                                if k0 >= qb:  # diagonal: causal triangle
                                    dcw = min(128, qw - c0)
                                    nc.vector.tensor_tensor(
                                        est[0:kw, c0:c0 + dcw],
                                        est[0:kw, c0:c0 + dcw],
                                        tri_sb[0:kw, 0:dcw], op=mult)
                                ests.append((est, kt, kw))
                            for (est, kt, kw) in ests:
                                c0i = max(0, kt * 128 - qb)
                                nc.tensor.matmul(zr[0:1, c0i:qw],
                                                 ones_b[0:kw, :],
                                                 est[0:kw, c0i:qw],
                                                 start=(kt == 0),
                                                 stop=(kt == nkt_b - 1))
                            for (est, kt, kw) in ests:
                                c0i = max(0, kt * 128 - qb)
                                nc.tensor.matmul(
                                    ot[:, c0i:qw],
                                    V_pre[si][0:kw, kt * HD + j * DH:
                                              kt * HD + j * DH + DH],
                                    est[0:kw, c0i:qw],
                                    start=(kt == 0), stop=(kt == nkt_b - 1))
                        recr = nrm_pool.tile([1, 512], bf16, tag="recr")
                        with nc.allow_low_precision(reason="1/Z scale in bf16"):
                            nc.vector.reciprocal(recr[0:1, 0:qw],
                                                 zr[0:1, 0:qw])
                        rb = ps_pool.tile([128, 512], fp32, tag="st", bufs=3, name="rb")
                        nc.tensor.matmul(rb[:, 0:qw], ones_rb[:],
                                         recr[0:1, 0:qw],
                                         start=True, stop=True)
                        rb_sb = nrm_pool.tile([128, 512], fp32, tag="rb")
                        nc.scalar.copy(rb_sb[:, 0:qw], rb[:, 0:qw])
                        g0 = q0 + qb
                        a = g0
                        while a < g0 + qw:
                            b_end = min(g0 + qw, (a // 128 + 1) * 128)
                            o0 = a - g0
                            cw = b_end - a
                            nc.vector.tensor_tensor(
                                attnT[j][a // 128][:, a % 128:a % 128 + cw],
                                ot[:, o0:o0 + cw], rb_sb[:, o0:o0 + cw],
                                op=mult)
                            a = b_end
                _flush_oproj(q1, late=(si == len(pre_ranges) - 1))

            if 'prefill' not in _ABLATE:
                for si, (q0, q1) in enumerate(pre_ranges):
                    V_pre[si] = v_block(q0, q1, f"v_pre{si}")
                    _emit_prefill(si, q0, q1)

            # ---- output projection: emitted per token tile (interleaved) ----
            if 'oproj' not in _ABLATE:
                for tt in sorted(_oproj_pending):
                    _emit_oproj(tt, late=True)
                _oproj_pending.clear()

    nc.compile()
    return nc


def _prep_inputs(x, w_q, w_k, w_v, w_o, k_cache, v_cache, nd, dec_idx):
    """Host-side shard prep: slice / transpose / tile / cast per core."""
    nt, hid = x.shape
    L = k_cache.shape[2]
    KHID = hid // 128
    HD = HPC * DH
    LT = L // 128

    xT = np.ascontiguousarray(x.T).astype(BF16)          # [hid, nt]
    tri = np.triu(np.ones((128, 128), np.float32)).astype(BF16)

    in_maps = []
    for c in range(NCORES):
        hd0 = c * HD
        m = {"xT": xT, "tri": tri}
        for name, w in (("wq_t", w_q), ("wk_t", w_k), ("wv_t", w_v)):
            ws = w[hd0:hd0 + HD, :].T.astype(BF16)        # [hid, HD]
            # tiled: [128, KHID*HD]; wt[p, k*HD+m] = ws[k*128+p, m]
            wt = np.ascontiguousarray(
                ws.reshape(KHID, 128, HD).transpose(1, 0, 2).reshape(
                    128, KHID * HD))
            m[name] = wt
        m["woT"] = np.ascontiguousarray(
            w_o[:, hd0:hd0 + HD].T).astype(BF16)          # [HD, hid]
        if nd > 0:
            kc = k_cache[dec_idx][:, 2 * c:2 * c + HPC]   # [nd, HPC, L, DH]
            m["ktc"] = np.ascontiguousarray(
                kc.transpose(0, 1, 3, 2)).astype(BF16)    # [nd,HPC,DH,L]
            vc = v_cache[dec_idx][:, 2 * c:2 * c + HPC]   # [nd, HPC, L, DH]
            m["vtc"] = np.ascontiguousarray(
                vc.reshape(len(dec_idx), HPC, LT, 128, DH)
                .transpose(0, 1, 3, 2, 4)).astype(BF16)   # [nd,HPC,128,LT,DH]
        in_maps.append(m)
    return in_maps


def kernel(x, w_q, w_k, w_v, w_o, k_cache, v_cache, n_decode,
           decode_sequence_lengths, decode_batch_idxs, n_prefill,
           prefill_lengths, prefill_batch_idxs):
    from concourse.bass_utils import run_bass_kernel_spmd

    x = np.asarray(x, np.float32)
    w_q = np.asarray(w_q, np.float32)
    w_k = np.asarray(w_k, np.float32)
    w_v = np.asarray(w_v, np.float32)
    w_o = np.asarray(w_o, np.float32)
    k_cache = np.asarray(k_cache, np.float32)
    v_cache = np.asarray(v_cache, np.float32)
    nd = int(n_decode)
    dec_lens = tuple(int(v) for v in np.asarray(decode_sequence_lengths)[:nd])
    dec_idx = np.asarray(decode_batch_idxs, np.int64)[:nd]
    plens = np.asarray(prefill_lengths, np.int64)

    nt, hid = x.shape
    L = k_cache.shape[2]
    T = nt - nd
    # prefill seq global-token ranges, clipped to the packed token count
    pre_ranges = []
    off = 0
    for ln in plens.tolist():
        if off >= T or ln <= 0:
            off += max(ln, 0)
            continue
        t0, t1 = off, min(off + ln, T)
        pre_ranges.append((nd + t0, nd + t1))
        off += ln
    if T > 0:
        if not pre_ranges:
            pre_ranges.append((nd, nd + T))
        elif pre_ranges[-1][1] < nd + T:
            # tokens beyond sum(prefill_lengths): jnp.searchsorted clamps
            # their seq id to the last sequence, so extend it
            pre_ranges[-1] = (pre_ranges[-1][0], nd + T)
    pre_ranges = tuple(pre_ranges)

    nc = _build_program(nt, hid, L, nd, dec_lens, pre_ranges)
    in_maps = _prep_inputs(x, w_q, w_k, w_v, w_o, k_cache, v_cache,
                           nd, dec_idx)
    res = run_bass_kernel_spmd(nc, in_maps, list(range(NCORES)))
    out = res.results[0]["out_partial"].astype(np.float64)
    for c in range(1, NCORES):
        out += res.results[c]["out_partial"]
    return out.astype(np.float32)

